# revision 7
# baseline (speedup 1.0000x reference)
"""Trainium2 Bass kernel for the CatNMF watermark loss (MSE + SSIM + BCE).

Contract: kernel(**inputs) takes FULL inputs
  cover  [16,3,512,512] f32
  wmed   [16,3,512,512] f32
  wm_orig[16,1024] f32
  wm_ext [16,1024] f32
  epoch  scalar int
and returns (total, ml, sv, wl) matching reference.py.

Strategy: pure data parallel over batch (2 images/core on 8 cores). Each core
computes partial sums (sum of squared diff, sum of ssim_map, sum of BCE terms);
the host combines them and applies the curriculum weights.

SSIM internals per core:
  - fields xp=x+y, xm=x-y, f4=xp^2-xm^2 (=4xy), s2=xp^2+xm^2 (=2(x^2+y^2))
  - separable 11-tap Gaussian smoothing done as banded matmuls on the PE
    (pass 1 contracts over H producing a transposed intermediate, pass 2
    contracts over W restoring orientation)
  - with sp=smooth(xp), sm=smooth(xm):
      2*N1 = sp^2 - sm^2 + 2C1          2*N2 = smooth(f4) + 2C2 - (2*N1) + 2C1
      2*D1 = sp^2 + sm^2 + 2C1          2*D2 = smooth(s2) + 2C2 - (2*D1) + 2C1
      ssim = (2N1*2N2) / (2D1*2D2)
"""

import sys

sys.path.insert(0, "/opt/trn_rl_repo")

import numpy as np

import concourse.bass as bass
import concourse.bacc as bacc
import concourse.mybir as mybir
from concourse.mybir import ActivationFunctionType as AF
from concourse.mybir import AluOpType as ALU
from concourse.tile import TileContext

F32 = mybir.dt.float32

# problem constants (must match reference.py)
B, C, H, W = 16, 3, 512, 512
NB = 1024
N_CORES = 8
B_LOC = B // N_CORES          # images per core
N_IMG = B_LOC * C             # channel-images per core
C1 = 0.01 ** 2
C2 = 0.03 ** 2
CURRICULUM_EP = 12
LI, LS, LW = 0.5, 0.8, 3.0

# band tiling of the 512x512 separable-conv operator
OFFS = [0, 123, 251, 379]
NS = [512, 138, 138, 133]


def _gauss_1d():
    coords = np.arange(11, dtype=np.float32) - 5
    g = np.exp(-(coords ** 2) / (2 * 1.5 ** 2)).astype(np.float32)
    g = g / g.sum()
    return g.astype(np.float32)


def _band_blocks():
    """4 blocks: block k maps h_in rows [128k,128k+128) -> h_out cols
    [OFFS[k], OFFS[k]+NS[k]). block0 is zero-padded to the full 512 columns so
    its start=True matmul initializes the whole PSUM bank."""
    g = _gauss_1d()
    blocks = []
    for k in range(4):
        blk = np.zeros((128, NS[k]), dtype=np.float32)
        for r in range(128):
            h_in = 128 * k + r
            for j in range(NS[k]):
                h_out = OFFS[k] + j
                d = h_in - h_out + 5
                if 0 <= d <= 10:
                    blk[r, j] = g[d]
        blocks.append(blk)
    return blocks


def _build_program(compile=True):
    nc = bacc.Bacc("TRN2", target_bir_lowering=False)

    # ---- DRAM I/O (per-core shard) ----
    cover = nc.declare_dram_parameter("cover", [B_LOC, C, H, W], F32, isOutput=False)
    wmed = nc.declare_dram_parameter("wmed", [B_LOC, C, H, W], F32, isOutput=False)
    wm_orig = nc.declare_dram_parameter("wm_orig", [B_LOC, NB], F32, isOutput=False)
    wm_ext = nc.declare_dram_parameter("wm_ext", [B_LOC, NB], F32, isOutput=False)
    bands = [
        nc.declare_dram_parameter(f"band{k}", [128, NS[k]], F32, isOutput=False)
        for k in range(4)
    ]
    out = nc.declare_dram_parameter("out", [1, 4], F32, isOutput=True)

    c11 = 2.0 * C1                 # constant for 2N1 / 2D1
    c12 = 2.0 * C1 + 2.0 * C2      # constant for 2N2 / 2D2

    with TileContext(nc) as tc:
        import contextlib

        with contextlib.ExitStack() as ctx:
            singles = ctx.enter_context(tc.tile_pool(name="singles", bufs=1))
            imgpool = ctx.enter_context(tc.tile_pool(name="img", bufs=2))
            fieldpool = ctx.enter_context(tc.tile_pool(name="field", bufs=1))
            itmpool = ctx.enter_context(tc.tile_pool(name="itm", bufs=1))
            cpool = ctx.enter_context(tc.tile_pool(name="pc", bufs=2))
            p1pool = ctx.enter_context(tc.tile_pool(name="psum1", bufs=2, space="PSUM"))
            p2pool = ctx.enter_context(tc.tile_pool(name="psum2", bufs=4, space="PSUM"))

            # band blocks + ones column (persistent)
            band_sb = []
            for k in range(4):
                t = singles.tile([128, NS[k]], F32, tag=f"band{k}")
                nc.sync.dma_start(out=t[:], in_=bands[k][:])
                band_sb.append(t)
            ones = singles.tile([128, 1], F32, tag="ones")
            nc.vector.memset(ones[:], 1.0)

            # accumulators (SBUF, fp32)
            acc_ml = singles.tile([128, N_IMG], F32, tag="acc_ml")
            acc_ss = singles.tile([128, 4 * N_IMG], F32, tag="acc_ss")
            acc_bce = singles.tile([128, 1], F32, tag="acc_bce")
            nc.vector.memset(acc_bce[:], 0.0)

            # ---------------- BCE (tiny; [2,1024] keeps the DMA on one queue) ----------------
            o_t = singles.tile([B_LOC, NB], F32, tag="wmo")
            e_t = singles.tile([B_LOC, NB], F32, tag="wme")
            nc.sync.dma_start(out=o_t[:], in_=wm_orig[:])
            nc.sync.dma_start(out=e_t[:], in_=wm_ext[:])
            l1 = singles.tile([B_LOC, NB], F32, tag="l1")
            l2 = singles.tile([B_LOC, NB], F32, tag="l2")
            om = singles.tile([B_LOC, NB], F32, tag="om")
            d12 = singles.tile([B_LOC, NB], F32, tag="d12")
            m1 = singles.tile([B_LOC, NB], F32, tag="m1")
            nc.scalar.activation(l1[:], e_t[:], AF.Ln)
            # om = 1 - e
            nc.vector.tensor_scalar(om[:], e_t[:], -1.0, 1.0, ALU.mult, ALU.add)
            nc.scalar.activation(l2[:], om[:], AF.Ln)
            nc.vector.tensor_tensor(d12[:], l1[:], l2[:], ALU.subtract)
            nc.vector.tensor_tensor(m1[:], o_t[:], d12[:], ALU.mult)
            # bce_elem = o*(l1-l2) + l2 ; acc_bce[0:2] = rowsum
            nc.vector.scalar_tensor_tensor(
                m1[:], m1[:], 0.0, l2[:], ALU.add, ALU.add,
                accum_out=acc_bce[:B_LOC, 0:1],
            )

            # ---------------- main per-image loop ----------------
            for img in range(N_IMG):
                b, ch = divmod(img, C)
                # SBUF image layout: [128, 4*512], partition p / free t*512+w
                # maps to pixel (t*128+p, w)
                x_t = imgpool.tile([128, 2048], F32, tag="x")
                y_t = imgpool.tile([128, 2048], F32, tag="y")
                src_x = wmed[b, ch].rearrange("(t p) w -> p t w", p=128)
                src_y = cover[b, ch].rearrange("(t p) w -> p t w", p=128)
                nc.sync.dma_start(out=x_t[:].rearrange("p (t w) -> p t w", t=4), in_=src_x)
                nc.sync.dma_start(out=y_t[:].rearrange("p (t w) -> p t w", t=4), in_=src_y)

                xp = fieldpool.tile([128, 2048], F32, tag="xp")
                xm = fieldpool.tile([128, 2048], F32, tag="xm")
                xp2 = fieldpool.tile([128, 2048], F32, tag="xp2")
                xm2 = fieldpool.tile([128, 2048], F32, tag="xm2")
                f4 = fieldpool.tile([128, 2048], F32, tag="f4")
                s2 = fieldpool.tile([128, 2048], F32, tag="s2")

                nc.vector.tensor_tensor(xp[:], x_t[:], y_t[:], ALU.add)
                nc.vector.tensor_tensor(xm[:], x_t[:], y_t[:], ALU.subtract)
                nc.scalar.activation(xp2[:], xp[:], AF.Square)
                # xm^2 with fused row-sum -> MSE partial
                nc.scalar.activation(
                    xm2[:], xm[:], AF.Square, accum_out=acc_ml[:, img : img + 1]
                )
                nc.vector.tensor_tensor(f4[:], xp2[:], xm2[:], ALU.subtract)
                nc.vector.tensor_tensor(s2[:], xp2[:], xm2[:], ALU.add)

                fields = [xp, xm, f4, s2]

                # ---- pass 1: contract over H -> interm[p=w(chunk m), m*512+h]
                interms = []
                for fi, F_t in enumerate(fields):
                    itm = itmpool.tile([128, 2048], F32, tag=f"itm{fi}")
                    interms.append(itm)
                    for m in range(4):
                        ps = p1pool.tile([128, 512], F32, tag="p1")
                        for k in range(4):
                            lhsT = F_t[:, k * 512 + m * 128 : k * 512 + m * 128 + 128]
                            nc.tensor.matmul(
                                ps[:, OFFS[k] : OFFS[k] + NS[k]],
                                lhsT,
                                band_sb[k][:],
                                start=(k == 0),
                                stop=(k == 3),
                            )
                        nc.any.tensor_copy(itm[:, m * 512 : (m + 1) * 512], ps[:])

                # ---- pass 2 (contract over W) + phase C per h-chunk t
                for t in range(4):
                    sm_ps = []
                    for fi, itm in enumerate(interms):
                        ps = p2pool.tile([128, 512], F32, tag="p2")
                        sm_ps.append(ps)
                        for k in range(4):
                            lhsT = itm[:, k * 512 + t * 128 : k * 512 + t * 128 + 128]
                            nc.tensor.matmul(
                                ps[:, OFFS[k] : OFFS[k] + NS[k]],
                                lhsT,
                                band_sb[k][:],
                                start=(k == 0),
                                stop=(k == 3),
                            )
                    sp_t, smm_t, sm4_t, ss2_t = sm_ps

                    A_t = cpool.tile([128, 512], F32, tag="A")
                    B_t = cpool.tile([128, 512], F32, tag="B")
                    n1 = cpool.tile([128, 512], F32, tag="n1")
                    d1 = cpool.tile([128, 512], F32, tag="d1")
                    n2 = cpool.tile([128, 512], F32, tag="n2")
                    d2 = cpool.tile([128, 512], F32, tag="d2")
                    num = cpool.tile([128, 512], F32, tag="num")
                    den = cpool.tile([128, 512], F32, tag="den")
                    rec = cpool.tile([128, 512], F32, tag="rec")
                    scr = cpool.tile([128, 512], F32, tag="scr")

                    nc.scalar.activation(A_t[:], sp_t[:], AF.Square)
                    nc.scalar.activation(B_t[:], smm_t[:], AF.Square)
                    # 2N1 = (A + 2C1) - B ; 2D1 = (A + 2C1) + B
                    nc.vector.scalar_tensor_tensor(
                        n1[:], A_t[:], c11, B_t[:], ALU.add, ALU.subtract
                    )
                    nc.vector.scalar_tensor_tensor(
                        d1[:], A_t[:], c11, B_t[:], ALU.add, ALU.add
                    )
                    # 2N2 = (SM4 + 2C1+2C2) - 2N1 ; 2D2 = (SS2 + 2C1+2C2) - 2D1
                    nc.vector.scalar_tensor_tensor(
                        n2[:], sm4_t[:], c12, n1[:], ALU.add, ALU.subtract
                    )
                    nc.vector.scalar_tensor_tensor(
                        d2[:], ss2_t[:], c12, d1[:], ALU.add, ALU.subtract
                    )
                    nc.gpsimd.tensor_tensor(num[:], n1[:], n2[:], ALU.mult)
                    nc.gpsimd.tensor_tensor(den[:], d1[:], d2[:], ALU.mult)
                    nc.vector.reciprocal_approx_fast(out=rec[:], in_=den[:])
                    col = 4 * img + t
                    nc.vector.scalar_tensor_tensor(
                        scr[:], num[:], 1.0, rec[:], ALU.mult, ALU.mult,
                        accum_out=acc_ss[:, col : col + 1],
                    )

            # ---------------- final reduction ----------------
            red = singles.tile([128, 3], F32, tag="red")
            nc.vector.reduce_sum(red[:, 0:1], acc_ml[:], axis=mybir.AxisListType.X)
            nc.vector.reduce_sum(red[:, 1:2], acc_ss[:], axis=mybir.AxisListType.X)
            nc.vector.tensor_copy(red[:, 2:3], acc_bce[:])
            ps_f = p1pool.tile([128, 512], F32, tag="p1")
            nc.tensor.matmul(ps_f[:1, 0:3], ones[:], red[:], start=True, stop=True)
            out_sb = singles.tile([1, 4], F32, tag="osb")
            nc.vector.memset(out_sb[:], 0.0)
            nc.vector.tensor_copy(out_sb[:, 0:3], ps_f[:1, 0:3])
            nc.sync.dma_start(out=out[:], in_=out_sb[:])

    if compile:
        nc.compile()
    return nc


_NC_CACHE = None


def _get_program():
    global _NC_CACHE
    if _NC_CACHE is None:
        _NC_CACHE = _build_program()
    return _NC_CACHE


def _make_in_maps(cover, wmed, wm_orig, wm_ext):
    blocks = _band_blocks()
    in_maps = []
    for c in range(N_CORES):
        sl = slice(c * B_LOC, (c + 1) * B_LOC)
        m = {
            "cover": np.ascontiguousarray(cover[sl]),
            "wmed": np.ascontiguousarray(wmed[sl]),
            "wm_orig": np.ascontiguousarray(wm_orig[sl]),
            "wm_ext": np.ascontiguousarray(wm_ext[sl]),
        }
        for k in range(4):
            m[f"band{k}"] = blocks[k]
        in_maps.append(m)
    return in_maps


def _combine(results, epoch):
    tot_ml = 0.0
    tot_ss = 0.0
    tot_bce = 0.0
    for r in results:
        v = np.asarray(r["out"], dtype=np.float64).reshape(-1)
        tot_ml += v[0]
        tot_ss += v[1]
        tot_bce += v[2]
    n_pix = float(B * C * H * W)
    ml = tot_ml / n_pix
    sv = tot_ss / n_pix
    wl = -tot_bce / float(B * NB)
    epoch = int(epoch)
    if epoch <= CURRICULUM_EP:
        w_img, w_ssim = 0.05, 0.05
    else:
        progress = min(1.0, (epoch - CURRICULUM_EP) / 10.0)
        w_img = 0.05 + (LI - 0.05) * progress
        w_ssim = 0.05 + (LS - 0.05) * progress
    total = w_img * ml + w_ssim * (1.0 - sv) + LW * wl
    return (
        np.float32(total),
        np.float32(ml),
        np.float32(sv),
        np.float32(wl),
    )


def kernel(cover, wmed, wm_orig, wm_ext, epoch):
    from concourse.bass_utils import run_bass_kernel_spmd

    nc = _get_program()
    in_maps = _make_in_maps(
        np.asarray(cover, dtype=np.float32),
        np.asarray(wmed, dtype=np.float32),
        np.asarray(wm_orig, dtype=np.float32),
        np.asarray(wm_ext, dtype=np.float32),
    )
    res = run_bass_kernel_spmd(nc, in_maps, core_ids=list(range(N_CORES)))
    return _combine(res.results, epoch)


if __name__ == "__main__":
    import reference

    inputs = reference.setup_inputs()
    inputs = {k: np.asarray(v) if k != "epoch" else v for k, v in inputs.items()}
    got = kernel(**inputs)
    print("kernel:", got)
